# revision 24
# baseline (speedup 1.0000x reference)
"""Multi-head attention Bass/Tile kernel for Trainium2 (8 NeuronCores).

Problem: B=32, NQ=NK=512, IN_DIM=512, H=8 heads, E=64, OUT_DIM=512, fp32.
Sharding: data-parallel over batch — 8 cores x 4 batches, all 8 heads local
per core, so no collectives are needed.

v2: bf16 matmul operands (FWL-eligible weight loads, half DMA/SBUF), and the
whole kernel is software-pipelined as 16 (batch, head-pair) units so the PE
never idles long enough for the HAM clock gate to re-throttle:

  unit(b, p):
    scoresT for heads 2p/2p+1 — the two K=64 matmuls sit on complementary
      array halves (base partitions 0/64 -> row-tiled, run concurrently)
    exp on ACT (scale=1/8), bf16 out
    AV for the previous pair: lhsT = Vaug[k, 65] = [V*(1-mask) | (1-mask)],
      so PSUM partition 64 of `up` is the softmax denominator for free
    woven projection matmuls to keep PE dense while ACT runs:
      p==0 -> V projection for batch b
      p==1 -> output projection for batch b-1
      p==2/3 -> Q/K projections for batch b+1
    denominator: reciprocal_approx_fast (DVE) -> ones[1,64].T @ rdr matmul
      broadcast; the two heads' broadcasts are col-tiled (out partitions
      0:64 / 64:128) and run concurrently
    Unorm = up[0:64] * bc  (DVE, bf16)

All PSUM flows through one [128, 1024] tag (2 banks x 4 bufs = all 8 banks).
"""

import os
import sys
import types

sys.path.insert(0, "/opt/trn_rl_repo")

import numpy as np

B, NQ, NK, DIN, H, E, DOUT = 32, 512, 512, 512, 8, 64, 512
NCORES = 8
BPC = B // NCORES   # batches per core
P = 128
C = DIN // P        # contraction chunks
T = NK // P         # k tiles
G = H // 2          # head-pair groups
EPAD = 64           # denominator pad block (matmul dst partitions: 32/64/128)
E1 = E + EPAD       # V columns per head incl. padded denominator section

_CACHE = {}
LAST_RESULT = None


def _round_f32r(a):
    """Round fp32 to float32r (RNE to 11 mantissa bits, low 12 bits zero)."""
    u = np.ascontiguousarray(a, np.float32).view(np.uint32)
    r = (u + np.uint32(0x7FF) + ((u >> np.uint32(12)) & np.uint32(1))) & np.uint32(
        0xFFFFF000
    )
    return r.view(np.float32)


def _install_ntff_hook():
    """trn_boot can't register the NTFF profile hook (antenv stub lacks
    axon_hooks); recreate the module so BASS_TRACE=1 profiling works."""
    if "antenv.axon_hooks" in sys.modules:
        return
    try:
        import antenv

        mod = types.ModuleType("antenv.axon_hooks")
        holder = [None]
        mod.set_axon_ntff_profile_hook = lambda h: holder.__setitem__(0, h)
        mod.get_axon_ntff_profile_hook = lambda: holder[0]
        sys.modules["antenv.axon_hooks"] = mod
        antenv.axon_hooks = mod
        if "/root/.axon_site" not in sys.path:
            sys.path.append("/root/.axon_site")
        from trn_agent_boot.trn_boot import _ntff_profile_via_ctypes

        mod.set_axon_ntff_profile_hook(
            _ntff_profile_via_ctypes("/opt/axon/libaxon_pjrt.so")
        )
    except Exception:
        pass


def _build():
    import concourse.bass as bass  # noqa: F401
    import concourse.mybir as mybir
    import concourse.tile as tile
    from concourse import bacc
    from concourse.dve_ops import (
        RECIP_APPROX_FAST_CONSTS as _rc,
        RECIPROCAL_APPROX_FAST as _rf,
    )

    f32 = mybir.dt.float32
    f32r = mybir.dt.float32r
    bf16 = mybir.dt.bfloat16

    nc = bacc.Bacc("TRN2", target_bir_lowering=False, debug=False,
                   num_devices=NCORES)

    qT = nc.dram_tensor("qT", [BPC, DIN, NQ], bf16, kind="ExternalInput")
    kT = nc.dram_tensor("kT", [BPC, DIN, NK], bf16, kind="ExternalInput")
    vT = nc.dram_tensor("vT", [BPC, DIN, NK], bf16, kind="ExternalInput")
    wq = nc.dram_tensor("wq", [DIN, H * E], bf16, kind="ExternalInput")
    wk = nc.dram_tensor("wk", [DIN, H * E], bf16, kind="ExternalInput")
    wv = nc.dram_tensor("wv", [DIN, H * E], bf16, kind="ExternalInput")
    wo = nc.dram_tensor("wo", [H * E, DOUT], bf16, kind="ExternalInput")
    mnot = nc.dram_tensor("mnot", [BPC, P, T], f32, kind="ExternalInput")
    onesr = nc.dram_tensor("onesr", [2, P], f32r, kind="ExternalInput")
    out = nc.dram_tensor("out", [BPC, NQ, DOUT], bf16, kind="ExternalOutput")

    with tile.TileContext(nc) as tc:
        with (
            tc.tile_pool(name="consts", bufs=1) as cpool,
            tc.tile_pool(name="io", bufs=2) as iopool,
            tc.tile_pool(name="work", bufs=2) as wpool,
            tc.tile_pool(name="ps", bufs=4, space="PSUM") as pspool,
        ):
            wq_sbs = [cpool.tile([P, H * E], bf16, name=f"wq{c}", tag=f"wq{c}") for c in range(C)]
            wk_sbs = [cpool.tile([P, H * E], bf16, name=f"wk{c}", tag=f"wk{c}") for c in range(C)]
            wv_sbs = [cpool.tile([P, H * E], bf16, name=f"wv{c}", tag=f"wv{c}") for c in range(C)]
            wo_sbs = [cpool.tile([P, DOUT], bf16, name=f"wo{c}", tag=f"wo{c}") for c in range(C)]
            ones_row = cpool.tile([2, P], f32r)
            wq_r = wq[:].rearrange("(c p) n -> c p n", p=P)
            wk_r = wk[:].rearrange("(c p) n -> c p n", p=P)
            wv_r = wv[:].rearrange("(c p) n -> c p n", p=P)
            wo_r = wo[:].rearrange("(c p) n -> c p n", p=P)

            def psum():
                return pspool.tile([P, 2 * NQ], f32, name="ps", tag="big")

            # per-batch SBUF state (rotated via tags)
            state = {}

            def dma_in(b):
                """Issue input DMAs for batch b; returns the SBUF tiles."""
                qT_sbs = [iopool.tile([P, NQ], bf16, name=f"qTc{c}", tag=f"qT{c}") for c in range(C)]
                kT_sbs = [iopool.tile([P, NK], bf16, name=f"kTc{c}", tag=f"kT{c}") for c in range(C)]
                vT_sb = iopool.tile([P, C, NK], bf16, name="vT_sb", tag="vT")
                mn_sb = iopool.tile([P, T], f32, name="mn_sb", tag="mn")
                qT_r = qT[b].rearrange("(c p) n -> c p n", p=P)
                kT_r = kT[b].rearrange("(c p) n -> c p n", p=P)
                if b == 0:
                    nc.sync.dma_start(ones_row[:], onesr[:])
                for c in range(C):
                    if b == 0:
                        nc.sync.dma_start(wq_sbs[c][:], wq_r[c])
                    nc.sync.dma_start(qT_sbs[c][:], qT_r[c])
                for c in range(C):
                    if b == 0:
                        nc.sync.dma_start(wk_sbs[c][:], wk_r[c])
                    nc.sync.dma_start(kT_sbs[c][:], kT_r[c])
                if b == 0:
                    for c in range(C):
                        nc.sync.dma_start(wv_sbs[c][:], wv_r[c])
                nc.sync.dma_start(vT_sb[:], vT[b].rearrange("(c p) n -> p c n", p=P))
                if b == 0:
                    for c in range(C):
                        nc.sync.dma_start(wo_sbs[c][:], wo_r[c])
                nc.sync.dma_start(mn_sb[:], mnot[b])
                state[("in", b)] = (qT_sbs, kT_sbs, vT_sb, mn_sb)

            _SENT = object()

            def gen_qk_group(b, g):
                """Q/K projection of head pair g, batch b (yield per matmul)."""
                qT_sbs, kT_sbs, _, _ = state[("in", b)]
                if g == 0:
                    state[("QT", b)] = wpool.tile([P, G, NQ], bf16, name="QT_sb", tag="QT")
                    state[("KT", b)] = wpool.tile([P, G, NK], bf16, name="KT_sb", tag="KT")
                QT_sb, KT_sb = state[("QT", b)], state[("KT", b)]
                gs = slice(g * P, (g + 1) * P)
                pqk = psum()
                for c in range(C):
                    nc.tensor.matmul(pqk[:, 0:NQ], lhsT=wq_sbs[c][:, gs],
                                     rhs=qT_sbs[c][:], start=(c == 0),
                                     stop=(c == C - 1), skip_group_check=True)
                    yield
                for c in range(C):
                    nc.tensor.matmul(pqk[:, NQ:2 * NQ], lhsT=wk_sbs[c][:, gs],
                                     rhs=kT_sbs[c][:], start=(c == 0),
                                     stop=(c == C - 1), skip_group_check=True)
                    yield
                yield
                nc.vector.tensor_copy(out=QT_sb[:, g, :], in_=pqk[:, 0:NQ])
                nc.vector.tensor_copy(out=KT_sb[:, g, :], in_=pqk[:, NQ:2 * NQ])

            def gen_vproj(b):
                """V projection -> Vaug[k, t, h*128 + {0:64 (1-m) | 64: V*(1-m)}]."""
                _, _, vT_sb, mn_sb = state[("in", b)]
                Vaug = wpool.tile([P, T, H * E1], bf16, name="Vaug", tag="Va")
                state[("Va", b)] = Vaug
                for half in range(2):
                    pv = psum()
                    for tt in range(2):
                        t = 2 * half + tt
                        for c in range(C):
                            nc.tensor.matmul(
                                pv[:, tt * H * E:(tt + 1) * H * E],
                                lhsT=vT_sb[:, c, t * P:(t + 1) * P],
                                rhs=wv_sbs[c][:], start=(c == 0),
                                stop=(c == C - 1), skip_group_check=True)
                            yield
                    yield
                    for tt in range(2):
                        t = 2 * half + tt
                        va_t = Vaug[:, t, :].rearrange("p (h e) -> p h e", e=E1)
                        nc.vector.tensor_scalar_mul(
                            va_t[:, :, EPAD:E1],
                            pv[:, tt * H * E:(tt + 1) * H * E].rearrange(
                                "p (h e) -> p h e", e=E),
                            mn_sb[:, t:t + 1])
                        nc.vector.tensor_copy(
                            out=va_t[:, :, 0:EPAD],
                            in_=mn_sb[:, t:t + 1, None].to_broadcast((P, H, EPAD)))
                    yield

            def gen_outproj(b, halves=(0, 1)):
                """Output projection of batch b from Unorm(b); DMA result."""
                Unorm = state[("Un", b)]
                for half in halves:
                    po = psum()
                    for tt in range(2):
                        qt = 2 * half + tt
                        for c in range(C):
                            nc.tensor.matmul(
                                po[:, tt * DOUT:(tt + 1) * DOUT],
                                lhsT=Unorm[:, c, qt * P:(qt + 1) * P],
                                rhs=wo_sbs[c][:], start=(c == 0),
                                stop=(c == C - 1), skip_group_check=True)
                            yield
                    yield
                    ob = iopool.tile([P, 2, DOUT], bf16, name="ob", tag="ob")
                    nc.vector.tensor_copy(
                        out=ob[:], in_=po[:].rearrange("p (t n) -> p t n", t=2))
                    nc.sync.dma_start(
                        out[b, 2 * half * P:(2 * half + 2) * P, :].rearrange(
                            "(t p) n -> p t n", p=P),
                        ob[:])

            def emit_norm(up, pb, pp):
                """reciprocal -> PE broadcast -> Unorm for pair (pb, pp)."""
                bc = psum()
                nc.tensor.matmul(bc[0:E, 0:NQ], lhsT=ones_row[0:1, 0:E],
                                 rhs=state["rdr"][0:1, 0:NQ],
                                 start=True, stop=True)
                nc.tensor.matmul(bc[0:E, NQ:2 * NQ], lhsT=ones_row[0:1, 0:E],
                                 rhs=state["rdr"][0:1, NQ:2 * NQ],
                                 start=True, stop=True)
                bcs = wpool.tile([P, NQ], f32, name="bcs", tag="bcs")
                nc.scalar.copy(out=bcs[0:E, :], in_=bc[0:E, 0:NQ])
                nc.scalar.copy(out=bcs[E:2 * E, :], in_=bc[0:E, NQ:2 * NQ])
                Un = state[("Un", pb)]
                nc.vector.tensor_mul(out=Un[0:E, pp, :],
                                     in0=up[EPAD:E1, 0:NQ], in1=bcs[0:E, :])
                nc.vector.tensor_mul(out=Un[E:2 * E, pp, :],
                                     in0=up[EPAD:E1, NQ:2 * NQ],
                                     in1=bcs[E:2 * E, :])

            def emit_recip(up):
                rdr = wpool.tile([1, 2 * NQ], f32r, name="rdr", tag="rdr")
                state["rdr"] = rdr
                nc.vector._custom_dve(_rf, out=rdr[:], in0=up[0:1, 0:2 * NQ],
                                      s0=_rc["s0"], s1=_rc["s1"],
                                      imm2=_rc["imm2"])

            def unit(b, p, prev, wovens):
                """scores+exp for (b, p); AV for prev pair; woven PE work."""
                import itertools
                wov = itertools.chain(*wovens)

                def pull(n):
                    for _ in range(n):
                        if next(wov, _SENT) is _SENT:
                            return

                QT_sb, KT_sb = state[("QT", b)], state[("KT", b)]
                exA = wpool.tile([P, T, NQ], bf16, name="exA", tag="ex", bufs=4)
                exB = wpool.tile([P, T, NQ], bf16, name="exB", tag="ex", bufs=4)
                state[("ex", b, p)] = (exA, exB)

                if prev is not None:
                    pb, pp = prev
                    exPA, exPB = state.pop(("ex", pb, pp))
                    VaP = state[("Va", pb)]
                    if pp == 0:
                        state[("Un", pb)] = wpool.tile(
                            [P, G, NQ], bf16, name="Unorm", tag="Un")

                pull(2)
                sc = [None, None]
                for thalf in range(2):
                    sc[0] = psum()  # head A, t pair
                    sc[1] = psum()  # head B, t pair
                    if thalf == 0 and prev is not None:
                        up = psum()
                    for tt in range(2):
                        t = 2 * thalf + tt
                        for hh in range(2):
                            es = slice(hh * E, (hh + 1) * E)
                            nc.tensor.matmul(
                                sc[hh][:, tt * NQ:(tt + 1) * NQ],
                                lhsT=KT_sb[es, p, t * P:(t + 1) * P],
                                rhs=QT_sb[es, p, :],
                                start=True, stop=True)
                        if prev is not None:
                            for hh, exP in ((0, exPA), (1, exPB)):
                                h = 2 * pp + hh
                                nc.tensor.matmul(
                                    up[0:E1, hh * NQ:(hh + 1) * NQ],
                                    lhsT=VaP[:, t, h * E1:(h + 1) * E1],
                                    rhs=exP[:, t, :],
                                    start=(t == 0), stop=(t == T - 1),
                                    skip_group_check=True)
                        if prev is not None and t == T - 1:
                            emit_recip(up)
                        pull(3)
                    for hh, ex in ((0, exA), (1, exB)):
                        nc.scalar.activation(
                            ex[:, 2 * thalf:2 * thalf + 2, :],
                            sc[hh][:].rearrange("p (t n) -> p t n", t=2),
                            mybir.ActivationFunctionType.Exp, scale=0.125)

                pull(999)
                if prev is not None:
                    emit_norm(up, pb, pp)

            # ---------------- pipeline ----------------
            dma_in(0)
            # HAM warm-up: redundant matmuls over the first-arriving chunks
            # (each start=True pass overwrites; none of this PSUM is read).
            wu = psum()
            for rep in range(16):
                nc.tensor.matmul(wu[0:P, 0:P], lhsT=ones_row[0:2, 0:P],
                                 rhs=ones_row[0:2, 0:P], start=True, stop=True,
                                 skip_group_check=True)
            for _ in gen_qk_group(0, 0):
                pass

            for b in range(BPC):
                for p in range(G):
                    prev = None
                    if not (b == 0 and p == 0):
                        prev = (b, p - 1) if p > 0 else (b - 1, G - 1)
                    wovens = []
                    if p == 0:
                        if b + 1 < BPC:
                            dma_in(b + 1)
                        if b == 0:
                            wovens = [gen_qk_group(0, 1), gen_vproj(0),
                                      gen_qk_group(0, 2), gen_qk_group(0, 3)]
                        else:
                            wovens.append(gen_vproj(b))
                    elif p == 1:
                        if b + 1 < BPC:
                            wovens.append(gen_qk_group(b + 1, 0))
                            wovens.append(gen_qk_group(b + 1, 1))
                        elif b > 0:
                            wovens.append(gen_outproj(b - 1, halves=(0,)))
                    elif p == 2:
                        if b > 0:
                            wovens.append(gen_outproj(
                                b - 1, halves=(1,) if b + 1 >= BPC else (0, 1)))
                    elif p == 3:
                        if b + 1 < BPC:
                            wovens.append(gen_qk_group(b + 1, 2))
                            wovens.append(gen_qk_group(b + 1, 3))
                    unit(b, p, prev, wovens)

            # ---- drain tail: AV+norm of the last pair interleaved with the
            # final output projection (c=0..2 first, c=3 after the norm) ----
            pb, pp = BPC - 1, G - 1
            exPA, exPB = state.pop(("ex", pb, pp))
            VaP = state[("Va", pb)]
            Unorm = state[("Un", pb)]
            up = psum()
            po01 = psum()
            po23 = psum()
            pos = [po01, po01, po23, po23]

            def op_mm(qt, c, start, stop):
                nc.tensor.matmul(
                    pos[qt][:, (qt % 2) * DOUT:(qt % 2 + 1) * DOUT],
                    lhsT=Unorm[:, c, qt * P:(qt + 1) * P],
                    rhs=wo_sbs[c][:], start=start, stop=stop,
                    skip_group_check=True)

            op_seq = [(qt, c) for c in range(C - 1) for qt in range(4)]
            oi = 0
            for t in range(T):
                for hh, exP in ((0, exPA), (1, exPB)):
                    h = 2 * pp + hh
                    nc.tensor.matmul(
                        up[0:E1, hh * NQ:(hh + 1) * NQ],
                        lhsT=VaP[:, t, h * E1:(h + 1) * E1],
                        rhs=exP[:, t, :],
                        start=(t == 0), stop=(t == T - 1),
                        skip_group_check=True)
                nc.tensor.matmul(wu[0:P, 0:P], lhsT=ones_row[0:2, 0:P],
                                 rhs=ones_row[0:2, 0:P], start=True, stop=True,
                                 skip_group_check=True)
                if t == T - 1:
                    emit_recip(up)
                for _ in range(3):
                    if oi < len(op_seq):
                        qt, c = op_seq[oi]
                        op_mm(qt, c, start=(c == 0), stop=False)
                        oi += 1
            while oi < len(op_seq):
                qt, c = op_seq[oi]
                op_mm(qt, c, start=(c == 0), stop=False)
                oi += 1
            emit_norm(up, pb, pp)
            for qt in range(4):
                op_mm(qt, C - 1, start=False, stop=True)
                obq = iopool.tile([P, DOUT], bf16, name="obq", tag="obq")
                nc.vector.tensor_copy(
                    out=obq[:],
                    in_=pos[qt][:, (qt % 2) * DOUT:(qt % 2 + 1) * DOUT])
                nc.sync.dma_start(out[pb, qt * P:(qt + 1) * P, :], obq[:])

    nc.compile()
    return nc


def kernel(q, k, v, mask, W_query, W_key, W_val, W_out):
    global LAST_RESULT
    _install_ntff_hook()
    import ml_dtypes
    from concourse.bass_utils import run_bass_kernel_spmd

    key = "nc_v2"
    if key not in _CACHE:
        _CACHE[key] = _build()
    nc = _CACHE[key]

    bf = ml_dtypes.bfloat16
    q = np.asarray(q, np.float32)
    k = np.asarray(k, np.float32)
    v = np.asarray(v, np.float32)
    wq_h = np.asarray(W_query, np.float32).transpose(1, 0, 2).reshape(DIN, H * E).astype(bf)
    wk_h = np.asarray(W_key, np.float32).transpose(1, 0, 2).reshape(DIN, H * E).astype(bf)
    wv_h = np.asarray(W_val, np.float32).transpose(1, 0, 2).reshape(DIN, H * E).astype(bf)
    wo_h = np.asarray(W_out, np.float32).reshape(H * E, DOUT).astype(bf)
    mn_full = (~np.asarray(mask, bool)).astype(np.float32)  # [B, NK]
    _ones2 = np.zeros((2, P), np.float32)
    _ones2[0, 0:E] = 1.0
    _ones2[1, E:2 * E] = 1.0

    in_maps = []
    for i in range(NCORES):
        sl = slice(i * BPC, (i + 1) * BPC)
        in_maps.append({
            "qT": np.ascontiguousarray(q[sl].transpose(0, 2, 1)).astype(bf),
            "kT": np.ascontiguousarray(k[sl].transpose(0, 2, 1)).astype(bf),
            "vT": np.ascontiguousarray(v[sl].transpose(0, 2, 1)).astype(bf),
            "wq": wq_h, "wk": wk_h, "wv": wv_h, "wo": wo_h,
            "onesr": _ones2,
            "mnot": np.ascontiguousarray(
                mn_full[sl].reshape(BPC, T, P).transpose(0, 2, 1)),
        })

    res = run_bass_kernel_spmd(nc, in_maps, core_ids=list(range(NCORES)))
    LAST_RESULT = res
    return np.concatenate(
        [r["out"].astype(np.float32) for r in res.results], axis=0)
